# revision 10
# baseline (speedup 1.0000x reference)
"""Trainium2 Bass kernel for nn_AiidkitTEAVGraphEmbedder (embedding lookup).

Sharding: data-parallel over nodes, 8 cores, tables replicated.

Positional encodings are computed on-chip per node tile:
    x = day * cturn[pair(k)]           (turns; DVE broadcast multiply)
    r = Identity(x + 2^23)             (ScalarE; magic-constant round)
    negf = (r - 2^23) - x              (DVE scalar_tensor_tensor; = -frac)
    even k: Sin(-2pi * negf)           (ScalarE LUT; arg in [-pi, pi])
    odd  k: 1 - 2*Sin(pi * negf)^2     (cos via double angle; ScalarE + DVE)

ent_attr rows are gathered with the batched GPSIMD dma_gather (int16 idx)
and added on DVE.  The 65536-row categ table gather has two modes:

  MODE "extgather": host folds ent_attr_table into categ_tables (weight
    preprocessing) and, because dma_gather indices are int16, partitions each
    32768-row output-position window's nodes into (id<32)/(id>=32) streams
    (stable order, padded); gathers hit fused[:32768] / fused[32768:]; rows
    return to natural positions via dma_scatter_add with window-local int16
    positions (padding idx -1 is trailing-trimmed by the ucode).

  MODE "indirect": natural order; comb = id*1024 + vocab computed on DVE;
    one [128,1]-offset indirect_dma_start per column with compute_op=add
    accumulating rows straight onto the PE tile (slower: ~1us/call on Pool).
"""
import numpy as np

MODE = "extgather"            # or "indirect"

N = 1_000_000
N_TABLES = 64
VOCAB = 1024
D = 128
D1 = 129
NCORES = 8
PER = N // NCORES              # 125000

# cont branch geometry
PCOLS = 992                    # 128*992 = 126976 padded nodes
CJ = 16                        # columns per cont chunk
NCH = PCOLS // CJ              # 62 chunks
CHN = 128 * CJ                 # 2048 nodes per chunk

# categ branch geometry (extgather mode)
# Output positions are processed in windows of WIN=32640 so that window-local
# scatter positions (and the junk slot 32767 used by padding) fit in int16.
# The output buffer gives each window a 32768-row span: rows [w*32768,
# w*32768+32640) are real positions w*32640.., the last 128 rows are junk.
WIN = 32640                    # real positions per window
WSPAN = 32768                  # buffer rows per window
JUNK = 32767                   # scatter idx used by padding lanes
NWIN = 4                       # 4*32640 = 130560 >= 125000
SCOLS = 132                    # columns per stream (16896 >= 16320 + 6.4 sd)
SN = 128 * SCOLS
NSTREAM = NWIN * 2
QCOLS = NSTREAM * SCOLS        # 1056
CJQ = 22
NCHQ_S = SCOLS // CJQ          # 6
NCHQ = NSTREAM * NCHQ_S        # 48
CHQ = 128 * CJQ                # 2816
OPAD = NWIN * WSPAN            # 131072

MAGIC = float(2 ** 23)
TWO_PI = float(2 * np.pi)
PI = float(np.pi)

_CACHE = {}


def _cturn_pairs(d):
    ne = (d + 1) // 2
    m = np.arange(ne)
    return (1.0 / (2 * np.pi * np.power(10000.0, (2 * m) / d))).astype(np.float32)


def _build(mode):
    from concourse import bass, bacc, mybir
    from concourse import tile

    f32, i32, i16 = mybir.dt.float32, mybir.dt.int32, mybir.dt.int16
    Alu = mybir.AluOpType
    Act = mybir.ActivationFunctionType

    nc = bacc.Bacc("TRN2", target_bir_lowering=False, debug=False,
                   num_devices=NCORES)

    qcols = QCOLS if mode == "extgather" else PCOLS
    cdays = nc.dram_tensor("cdays", [128, PCOLS], i32, kind="ExternalInput").ap()
    cvals = nc.dram_tensor("cvals", [128, PCOLS], f32, kind="ExternalInput").ap()
    cids16 = nc.dram_tensor("cids16", [16, 128 * PCOLS // 16], i16,
                            kind="ExternalInput").ap()
    qdays = nc.dram_tensor("qdays", [128, qcols], i32, kind="ExternalInput").ap()
    if mode == "extgather":
        qvidx = nc.dram_tensor("qvidx", [16, 128 * QCOLS // 16], i16,
                               kind="ExternalInput").ap()
        qpos = nc.dram_tensor("qpos", [16, 128 * QCOLS // 16], i16,
                              kind="ExternalInput").ap()
    else:
        qids = nc.dram_tensor("qids", [128, PCOLS], i32, kind="ExternalInput").ap()
        qvoc = nc.dram_tensor("qvoc", [128, PCOLS], i32, kind="ExternalInput").ap()
        qids16 = nc.dram_tensor("qids16", [16, 128 * PCOLS // 16], i16,
                                kind="ExternalInput").ap()
    ea = nc.dram_tensor("ea", [N_TABLES, D], f32, kind="ExternalInput").ap()
    qtabf = nc.dram_tensor("qtabf", [N_TABLES * VOCAB, D], f32,
                           kind="ExternalInput").ap()
    cc65 = nc.dram_tensor("cc65", [65], f32, kind="ExternalInput").ap()
    cc64 = nc.dram_tensor("cc64", [64], f32, kind="ExternalInput").ap()
    ocont = nc.dram_tensor("ocont", [128 * PCOLS, D1], f32,
                           kind="ExternalOutput").ap()
    opad = OPAD if mode == "extgather" else 128 * PCOLS
    ocateg = nc.dram_tensor("ocateg", [opad, D], f32, kind="ExternalOutput").ap()

    ocont_t = ocont.rearrange("(j p) d -> p j d", p=128)
    if mode == "indirect":
        ocateg_t = ocateg.rearrange("(j p) d -> p j d", p=128)

    def bcast_mid(t, w, d):
        a = t[:]
        return bass.AP(a.tensor, a.offset, [list(a.ap[0]), [0, w], [1, d]])

    def day_bc(t, w, d):
        a = t[:]
        return bass.AP(a.tensor, a.offset, [list(a.ap[0]), list(a.ap[1]), [0, d]])

    def stride2(a, start, cnt):
        return bass.AP(a.tensor, a.offset + start,
                       [list(a.ap[0]), list(a.ap[1]), [2, cnt]])

    def sub0(a, cnt):
        return bass.AP(a.tensor, a.offset,
                       [list(a.ap[0]), list(a.ap[1]), [1, cnt]])

    with tile.TileContext(nc) as tc:
        with tc.tile_pool(name="sbuf", bufs=2) as pool:
            cc65t = pool.tile([128, 65], f32, tag="cct", name="cc65t")
            cc64t = pool.tile([128, 64], f32, tag="cct", name="cc64t")
            nc.sync.dma_start(cc65t[:], cc65[None, :].to_broadcast([128, 65]))
            nc.sync.dma_start(cc64t[:], cc64[None, :].to_broadcast([128, 64]))
            magic = pool.tile([128, 1], f32, tag="magic")
            nc.vector.memset(magic[:], MAGIC)

            # resident wrapped-16 index tiles (replicated across partitions)
            cidx_t = pool.tile([128, 128 * PCOLS // 16], i16, tag="cidx_t")
            for g in range(8):
                nc.sync.dma_start(cidx_t[16 * g:16 * (g + 1), :], cids16[:, :])
            if mode == "extgather":
                vidx_t = pool.tile([128, 128 * QCOLS // 16], i16, tag="vidx_t")
                pos_t = pool.tile([128, 128 * QCOLS // 16], i16, tag="pos_t")
                for g in range(8):
                    nc.scalar.dma_start(vidx_t[16 * g:16 * (g + 1), :], qvidx[:, :])
                    nc.scalar.dma_start(pos_t[16 * g:16 * (g + 1), :], qpos[:, :])
            else:
                qidx_t = pool.tile([128, 128 * PCOLS // 16], i16, tag="vidx_t")
                for g in range(8):
                    nc.scalar.dma_start(qidx_t[16 * g:16 * (g + 1), :], qids16[:, :])

            def pe_encode(pe_ap, dayt, w, d, cct):
                ne = (d + 1) // 2
                no = d // 2
                dayf = pool.tile([128, w], f32, tag="dayf", name="dayf")
                nc.vector.tensor_copy(out=dayf[:], in_=dayt[:])
                prod = pool.tile([128, w, ne], f32, tag="prod", name="prod")
                rnd = pool.tile([128, w, ne], f32, tag="rnd", name="rnd")
                nc.vector.tensor_tensor(out=prod[:], in0=day_bc(dayf, w, ne),
                                        in1=bcast_mid(cct, w, ne), op=Alu.mult)
                nc.scalar.activation(out=rnd[:], in_=prod[:], func=Act.Identity,
                                     bias=magic[:], scale=1.0)
                nc.vector.scalar_tensor_tensor(out=rnd[:], in0=rnd[:],
                                               scalar=MAGIC, in1=prod[:],
                                               op0=Alu.subtract, op1=Alu.subtract)
                nc.scalar.activation(out=stride2(pe_ap, 0, ne), in_=rnd[:],
                                     func=Act.Sin, bias=0.0, scale=-TWO_PI)
                s = prod
                nc.scalar.activation(out=sub0(s[:], no), in_=sub0(rnd[:], no),
                                     func=Act.Sin, bias=0.0, scale=PI)
                nc.scalar.activation(out=sub0(s[:], no), in_=sub0(s[:], no),
                                     func=Act.Square, bias=0.0, scale=1.0)
                nc.vector.tensor_scalar(out=stride2(pe_ap, 1, no),
                                        in0=sub0(s[:], no), scalar1=-2.0,
                                        scalar2=1.0, op0=Alu.mult, op1=Alu.add)

            # ---------------- continuous branch ----------------
            for ch in range(NCH):
                j0 = ch * CJ
                w16 = ch * (CHN // 16)
                cday = pool.tile([128, CJ], i32, tag="day32", name="cday")
                cval = pool.tile([128, CJ], f32, tag="cval", name="cval")
                nc.sync.dma_start(cday[:], cdays[:, j0:j0 + CJ])
                nc.sync.dma_start(cval[:], cvals[:, j0:j0 + CJ])

                pec = pool.tile([128, CJ, D1], f32, tag="pe", name="pec")
                pe_encode(pec[:], cday, CJ, D1, cc65t)

                eac = pool.tile([128, CJ, D], f32, tag="gat", name="eac")
                nc.gpsimd.dma_gather(
                    out_ap=eac[:], in_ap=ea[:],
                    idxs_ap=cidx_t[:, w16:w16 + CHN // 16],
                    num_idxs=CHN, num_idxs_reg=CHN, elem_size=D, single_packet=False)
                nc.vector.tensor_tensor(out=sub0(pec[:], D), in0=sub0(pec[:], D),
                                        in1=eac[:], op=Alu.add)
                pccol = bass.AP(pec[:].tensor, pec[:].offset + D,
                                [list(pec[:].ap[0]), list(pec[:].ap[1]), [1, 1]])
                vcol = bass.AP(cval[:].tensor, cval[:].offset,
                               [list(cval[:].ap[0]), list(cval[:].ap[1]), [0, 1]])
                nc.vector.tensor_tensor(out=pccol, in0=pccol, in1=vcol, op=Alu.add)
                nc.sync.dma_start(ocont_t[:, j0:j0 + CJ, :], pec[:])

            # ---------------- categorical branch ----------------
            if mode == "extgather":
                for ch in range(NCHQ):
                    stream = ch // NCHQ_S
                    win, hi = stream // 2, stream % 2
                    j0 = ch * CJQ
                    w16 = ch * (CHQ // 16)

                    qday = pool.tile([128, CJQ], i32, tag="day32", name="qday")
                    nc.sync.dma_start(qday[:], qdays[:, j0:j0 + CJQ])
                    peq = pool.tile([128, CJQ, D], f32, tag="pe", name="peq")
                    pe_encode(peq[:], qday, CJQ, D, cc64t)

                    vt = pool.tile([128, CJQ, D], f32, tag="gat", name="vt")
                    HALF = N_TABLES * VOCAB // 2
                    src = qtabf[HALF:, :] if hi else qtabf[:HALF, :]
                    nc.gpsimd.dma_gather(
                        out_ap=vt[:], in_ap=src,
                        idxs_ap=vidx_t[:, w16:w16 + CHQ // 16],
                        num_idxs=CHQ, num_idxs_reg=CHQ, elem_size=D, single_packet=False)
                    nc.vector.tensor_tensor(out=peq[:], in0=peq[:], in1=vt[:],
                                            op=Alu.add)
                    nc.gpsimd.dma_scatter_add(
                        out_ap=ocateg[win * WSPAN:(win + 1) * WSPAN, :],
                        in_ap=peq[:],
                        idxs_ap=pos_t[:, w16:w16 + CHQ // 16],
                        num_idxs=CHQ, num_idxs_reg=CHQ, elem_size=D, single_packet=False)
            else:
                for ch in range(NCH):
                    j0 = ch * CJ
                    w16 = ch * (CHN // 16)
                    qday = pool.tile([128, CJ], i32, tag="day32", name="qday")
                    qidt = pool.tile([128, CJ], i32, tag="qidt", name="qidt")
                    qvot = pool.tile([128, CJ], i32, tag="qvot", name="qvot")
                    nc.sync.dma_start(qday[:], qdays[:, j0:j0 + CJ])
                    nc.sync.dma_start(qidt[:], qids[:, j0:j0 + CJ])
                    nc.sync.dma_start(qvot[:], qvoc[:, j0:j0 + CJ])
                    comb = pool.tile([128, CJ], i32, tag="comb", name="comb")
                    nc.vector.tensor_scalar(out=comb[:], in0=qidt[:],
                                            scalar1=VOCAB, scalar2=None,
                                            op0=Alu.mult)
                    nc.vector.tensor_tensor(out=comb[:], in0=comb[:],
                                            in1=qvot[:], op=Alu.add)

                    peq = pool.tile([128, CJ, D], f32, tag="pe", name="peq")
                    pe_encode(peq[:], qday, CJ, D, cc64t)
                    eaq = pool.tile([128, CJ, D], f32, tag="gat", name="eaq")
                    nc.gpsimd.dma_gather(
                        out_ap=eaq[:], in_ap=ea[:],
                        idxs_ap=qidx_t[:, w16:w16 + CHN // 16],
                        num_idxs=CHN, num_idxs_reg=CHN, elem_size=D, single_packet=False)
                    nc.vector.tensor_tensor(out=peq[:], in0=peq[:], in1=eaq[:],
                                            op=Alu.add)
                    for w in range(CJ):
                        nc.gpsimd.indirect_dma_start(
                            out=peq[:, w, :], out_offset=None, in_=qtabf,
                            in_offset=bass.IndirectOffsetOnAxis(
                                ap=comb[:, w:w + 1], axis=0),
                            compute_op=Alu.add)
                    nc.sync.dma_start(ocateg_t[:, j0:j0 + CJ, :], peq[:])

    nc.finalize()
    return nc


def _get_nc():
    key = f"nc_{MODE}"
    if key not in _CACHE:
        _CACHE[key] = _build(MODE)
    return _CACHE[key]


def _wrap16(a, chunk):
    n = a.shape[0]
    assert n % chunk == 0
    blocks = a.reshape(n // chunk, chunk // 16, 16)
    return np.concatenate(
        [blocks[c].T for c in range(n // chunk)], axis=1).astype(np.int16)


def _tcols(a, dtype):
    return np.ascontiguousarray(a.reshape(-1, 128).T).astype(dtype, copy=False)


def kernel(cont_ent_attr_ids, cont_vals, cont_days,
           categ_ent_attr_ids, categ_vocab_ids, categ_days,
           ent_attr_table, categ_tables):
    from concourse import bass_utils

    nc = _get_nc()
    ea = np.ascontiguousarray(ent_attr_table, np.float32)
    if MODE == "extgather":
        qtabf = (np.asarray(categ_tables, np.float32)
                 + ea[:, None, :]).reshape(N_TABLES * VOCAB, D)
    else:
        qtabf = np.ascontiguousarray(
            np.asarray(categ_tables, np.float32).reshape(N_TABLES * VOCAB, D))
    cc65 = _cturn_pairs(D1)
    cc64 = _cturn_pairs(D)

    cont_ids = np.asarray(cont_ent_attr_ids, np.int32)
    cont_vals = np.asarray(cont_vals, np.float32)
    cont_days = np.asarray(cont_days, np.int32)
    q_ids = np.asarray(categ_ent_attr_ids, np.int32)
    q_voc = np.asarray(categ_vocab_ids, np.int32)
    q_days = np.asarray(categ_days, np.int32)

    in_maps = []
    for c in range(NCORES):
        s = slice(c * PER, (c + 1) * PER)

        def padc(a):
            return np.concatenate([a, np.zeros(128 * PCOLS - PER, a.dtype)])

        m = dict(
            cdays=_tcols(padc(cont_days[s]), np.int32),
            cvals=_tcols(padc(cont_vals[s]), np.float32),
            cids16=_wrap16(padc(cont_ids[s]).astype(np.int16), CHN),
            ea=ea, qtabf=qtabf, cc65=cc65, cc64=cc64,
        )
        qi, qv, qd = q_ids[s], q_voc[s], q_days[s]
        if MODE == "extgather":
            comb = qi * VOCAB + qv
            sdays = np.zeros(128 * QCOLS, np.int32)
            svidx = np.zeros(128 * QCOLS, np.int16)   # padding gathers row 0
            spos = np.full(128 * QCOLS, JUNK, np.int16)  # padding -> junk rows
            HALF = N_TABLES * VOCAB // 2              # 32768 table rows/half
            for win in range(NWIN):
                base = win * WIN
                end = min(base + WIN, PER)
                wsl = slice(base, end)
                wcomb, wday = comb[wsl], qd[wsl]
                wpos = np.arange(end - base, dtype=np.int32)
                lo = wcomb < HALF
                for hi in (0, 1):
                    stream = win * 2 + hi
                    sel = ~lo if hi else lo
                    k = int(sel.sum())
                    assert k <= SN, f"stream overflow {k} > {SN}"
                    o0 = stream * SN
                    sdays[o0:o0 + k] = wday[sel]
                    svidx[o0:o0 + k] = (wcomb[sel] - (HALF if hi else 0)
                                        ).astype(np.int16)
                    spos[o0:o0 + k] = wpos[sel].astype(np.int16)
            m.update(qdays=_tcols(sdays, np.int32),
                     qvidx=_wrap16(svidx, CHQ), qpos=_wrap16(spos, CHQ))
        else:
            m.update(qdays=_tcols(padc(qd), np.int32),
                     qids=_tcols(padc(qi), np.int32),
                     qvoc=_tcols(padc(qv), np.int32),
                     qids16=_wrap16(padc(qi).astype(np.int16), CHN))
        in_maps.append(m)

    res = bass_utils.run_bass_kernel_spmd(
        nc, in_maps, core_ids=list(range(NCORES)), **_CACHE.get("run_kwargs", {}))
    _CACHE["last_res"] = res

    cont = np.concatenate(
        [res.results[c]["ocont"][:PER] for c in range(NCORES)])
    if MODE == "extgather":
        categ = np.concatenate(
            [np.concatenate([res.results[c]["ocateg"][w * WSPAN:
                                                      w * WSPAN + WIN]
                             for w in range(NWIN)])[:PER]
             for c in range(NCORES)])
    else:
        categ = np.concatenate(
            [res.results[c]["ocateg"][:PER] for c in range(NCORES)])
    return cont, categ
